# revision 1
# baseline (speedup 1.0000x reference)
"""Trainium2 Bass kernel for DevConv-style GNN message passing.

Reference computation:
    rel_t = (x[row] - x[col]) @ W_theta.T          # [E, 128]
    aggr  = segment_max(rel_t, row, N)             # [N, 128], empty -> 0
    out   = aggr @ W_phi.T                         # [N, 128]

Key reformulation: with y = x @ W_theta.T, within a segment (fixed row d)
    max_e (y[d] - y[col_e]) = y[d] - min_e y[col_e]     (per channel)
so the per-edge matmul disappears and only ONE gather per edge (y[col]) is
needed, followed by a segmented min.

Distribution: nodes are assigned to the 8 cores by degree-rank striping
(rank r -> core r % 8). Each core:
  Phase A: computes y = x @ W_theta.T for ALL nodes (bf16) into an HBM
           table that is split into 4 chunks of <=32767 rows (+1 sentinel
           row of +3e38 per chunk) because dma_gather indices are int16.
  Phase B: for each 128-node tile and each chunk, gathers y[col] rows into
           a padded [128 nodes x B slots] SBUF rect via dma_gather
           (pad slots point at the chunk sentinel), then pairwise-min folds
           the slots and merges chunks -> m[d] = min_e y[col_e].
  Phase C: aggr = y_own - m  (y_own computed on-chip from x_own),
           out_tile = aggr @ W_phi.T via PE transpose + matmul.
Host un-permutes the concatenated core outputs and zeroes empty nodes.
"""
import sys
import os

sys.path.insert(0, "/opt/trn_rl_repo")

from contextlib import ExitStack
from dataclasses import dataclass

import numpy as np
import ml_dtypes

import concourse.bass as bass
import concourse.tile as tile
from concourse import bacc, mybir
from concourse.masks import make_identity

import time

import jax
from jax.sharding import Mesh, PartitionSpec
from jax.experimental.shard_map import shard_map

from concourse.bass2jax import (
    _bass_exec_p, install_neuronx_cc_hook, partition_id_tensor)


class BassRunner:
    """Keeps a jitted PJRT executable for a Bass program so it can be run
    repeatedly on device-resident inputs (for wall-clock timing)."""

    def __init__(self, nc, n_cores: int):
        install_neuronx_cc_hook()
        self.nc = nc
        self.n_cores = n_cores
        partition_name = nc.partition_id_tensor.name if nc.partition_id_tensor else None
        in_names, out_names, out_avals = [], [], []
        for alloc in nc.m.functions[0].allocations:
            if not isinstance(alloc, mybir.MemoryLocationSet):
                continue
            name = alloc.memorylocations[0].name
            if alloc.kind == "ExternalInput":
                if name != partition_name:
                    in_names.append(name)
            elif alloc.kind == "ExternalOutput":
                out_names.append(name)
                out_avals.append(jax.core.ShapedArray(
                    tuple(alloc.tensor_shape), mybir.dt.np(alloc.dtype)))
        self.in_names, self.out_names, self.out_avals = in_names, out_names, out_avals
        self.n_params = len(in_names)
        all_in_names = list(in_names) + list(out_names)
        if partition_name is not None:
            all_in_names.append(partition_name)

        def _body(*args):
            operands = list(args)
            if partition_name is not None:
                operands.append(partition_id_tensor())
            outs = _bass_exec_p.bind(
                *operands,
                out_avals=tuple(out_avals),
                in_names=tuple(all_in_names),
                out_names=tuple(out_names),
                lowering_input_output_aliases=(),
                sim_require_finite=True,
                sim_require_nnan=True,
                nc=nc,
            )
            return tuple(outs)

        devices = jax.devices()[:n_cores]
        self.mesh = Mesh(np.asarray(devices), ("core",))
        n_outs = len(out_names)
        in_specs = (PartitionSpec("core"),) * (self.n_params + n_outs)
        out_specs = (PartitionSpec("core"),) * n_outs
        self.fn = jax.jit(
            shard_map(_body, mesh=self.mesh, in_specs=in_specs,
                      out_specs=out_specs, check_rep=False),
            keep_unused=True,
        )
        self._dev_args = None

    def prepare(self, in_maps):
        assert len(in_maps) == self.n_cores
        concat_in = [
            np.concatenate([np.asarray(in_maps[c][n]) for c in range(self.n_cores)],
                           axis=0)
            for n in self.in_names
        ]
        concat_zeros = [
            np.zeros((self.n_cores * a.shape[0], *a.shape[1:]), a.dtype)
            for a in self.out_avals
        ]
        sharding = jax.sharding.NamedSharding(self.mesh, PartitionSpec("core"))
        self._dev_args = [jax.device_put(v, sharding) for v in concat_in + concat_zeros]
        return self

    def run(self):
        outs = self.fn(*self._dev_args)
        jax.block_until_ready(outs)
        return outs

    def results(self, outs):
        return [
            {n: np.asarray(outs[i]).reshape(self.n_cores, *self.out_avals[i].shape)[c]
             for i, n in enumerate(self.out_names)}
            for c in range(self.n_cores)
        ]

    def time_ns(self, iters=5, warmup=2):
        for _ in range(warmup):
            self.run()
        ts = []
        for _ in range(iters):
            t0 = time.perf_counter()
            self.run()
            ts.append((time.perf_counter() - t0) * 1e9)
        return min(ts)


CH = 128
F32 = mybir.dt.float32
BF16 = mybir.dt.bfloat16
I16 = mybir.dt.int16
SENT_VAL = 3.0e38
IDX_PER_INST = 1024          # dma_gather crashes >= 2048 idx/instruction
BLK_PER_INST = IDX_PER_INST // 128


@dataclass(frozen=True)
class Cfg:
    N: int = 100_000
    E: int = 3_200_000
    n_cores: int = 8
    chunk_real: int = 32_767     # rows addressable by int16 (sentinel at chunk_real)

    @property
    def n_chunks(self):
        return (self.N + self.chunk_real - 1) // self.chunk_real

    @property
    def chunk_stride(self):
        return self.chunk_real + 1

    @property
    def npc(self):
        assert self.N % self.n_cores == 0
        return self.N // self.n_cores

    @property
    def tiles(self):
        return (self.npc + 127) // 128


def _wrap16(seg: np.ndarray) -> np.ndarray:
    """Per-instruction idx wrap: flat [n] -> [128, n//16]; idx i at
    (partition i%16, col i//16), replicated across the 8 gpsimd groups."""
    n = seg.shape[-1]
    w = seg.reshape(*seg.shape[:-1], n // 16, 16)
    w = np.swapaxes(w, -1, -2)                       # [..., 16, n//16]
    return np.tile(w, (1,) * (seg.ndim - 1) + (8, 1))


def prep(x, edge_index, cfg: Cfg):
    """Host-side data prep. Returns (plan, per-core inputs, unpermute info)."""
    N, E, NC = cfg.N, cfg.E, cfg.n_cores
    CR, NK, T = cfg.chunk_real, cfg.n_chunks, cfg.tiles
    row = np.asarray(edge_index[0], dtype=np.int64)
    col = np.asarray(edge_index[1], dtype=np.int64)

    deg = np.bincount(row, minlength=N)
    order = np.argsort(-deg, kind="stable")          # node ids by desc degree
    core_of = np.empty(N, np.int64)
    pos_of = np.empty(N, np.int64)
    r = np.arange(N)
    core_of[order] = r % NC
    pos_of[order] = r // NC

    ec = core_of[row]
    ep = pos_of[row]
    ek = col // CR
    elocal = (col - ek * CR).astype(np.int16)
    et = ep // 128
    ed = ep % 128

    # per-(core,tile,chunk,node) counts and within-group slot index j
    key = ((ec * T + et) * NK + ek) * 128 + ed
    o = np.argsort(key, kind="stable")
    ks = key[o]
    first = np.r_[True, ks[1:] != ks[:-1]]
    run_id = np.cumsum(first) - 1
    run_start = np.flatnonzero(first)
    j = np.arange(E) - run_start[run_id]

    cnt = np.bincount(key, minlength=NC * T * NK * 128).reshape(NC, T, NK, 128)
    B = cnt.max(axis=(0, 3)).astype(np.int64)        # [T, NK] shared structure

    Bf = B.reshape(-1)
    off = np.concatenate([[0], np.cumsum(Bf * 128)])  # slot offset per (t,k)
    total_slots = int(off[-1])

    idx_all = np.full((NC, total_slots), CR, np.int16)   # sentinel local idx
    tk = et[o] * NK + ek[o]
    pos_in = off[tk] + j * 128 + ed[o]
    idx_all[ec[o], pos_in] = elocal[o]

    # split into gather instructions and build wrapped idx input
    insts = []           # (t, k, g0blk, nblk, col_off)
    tile_cols = []       # per tile: (col_start, col_end)
    wsegs = []
    col_off = 0
    for t in range(T):
        t_start = col_off
        for k in range(NK):
            btk = int(B[t, k])
            base = int(off[t * NK + k])
            for g0 in range(0, btk, BLK_PER_INST):
                nb = min(BLK_PER_INST, btk - g0)
                n_i = nb * 128
                seg = idx_all[:, base + g0 * 128: base + g0 * 128 + n_i]
                wsegs.append(_wrap16(seg))
                insts.append((t, k, g0, nb, col_off))
                col_off += n_i // 16
        tile_cols.append((t_start, col_off))
    idxw = np.concatenate(wsegs, axis=2) if wsegs else np.zeros((NC, 128, 0), np.int16)
    W_total = idxw.shape[2]

    # per-core x_own in pos order, padded to T*128 rows
    own_nodes = np.empty((NC, cfg.npc), np.int64)
    own_nodes[core_of[order], pos_of[order]] = order  # own_nodes[c, p] = node id
    x_np = np.asarray(x, dtype=np.float32)
    x_own = np.zeros((NC, T * 128, CH), np.float32)
    x_own[:, : cfg.npc] = x_np[own_nodes]

    plan = dict(cfg=cfg, B=B, insts=insts, tile_cols=tile_cols, W_total=W_total)
    return plan, idxw, x_own, own_nodes, deg


def build_program(plan, reps=1, phases="abc"):
    cfg: Cfg = plan["cfg"]
    N, NK, CR, T = cfg.N, cfg.n_chunks, cfg.chunk_real, cfg.tiles
    CS = cfg.chunk_stride
    B, insts, tile_cols, W_total = (
        plan["B"], plan["insts"], plan["tile_cols"], plan["W_total"])

    nc = bacc.Bacc(None, target_bir_lowering=False, num_swdge_queues=4)
    x_full = nc.declare_dram_parameter("x_full", [N, CH], F32, isOutput=False)
    x_own = nc.declare_dram_parameter("x_own", [T * 128, CH], F32, isOutput=False)
    wth = nc.declare_dram_parameter("w_theta_t", [CH, CH], F32, isOutput=False)
    wph = nc.declare_dram_parameter("w_phi_t", [CH, CH], F32, isOutput=False)
    idxw = nc.declare_dram_parameter("idxw", [128, max(W_total, 16)], I16, isOutput=False)
    out = nc.declare_dram_parameter("out", [T * 128, CH], F32, isOutput=True)

    qc = [0]  # gather queue rotation

    with tile.TileContext(nc) as tc:
        with ExitStack() as ctx:
            consts = ctx.enter_context(tc.tile_pool(name="consts", bufs=1))
            dram = ctx.enter_context(tc.tile_pool(name="dram", bufs=1, space="DRAM"))
            ax = ctx.enter_context(tc.tile_pool(name="ax", bufs=2))
            axT = ctx.enter_context(tc.tile_pool(name="axT", bufs=2))
            ay = ctx.enter_context(tc.tile_pool(name="ay", bufs=2))
            ps_t = ctx.enter_context(tc.tile_pool(name="ps_t", bufs=2, space="PSUM"))
            ps_y = ctx.enter_context(tc.tile_pool(name="ps_y", bufs=2, space="PSUM"))
            ps_c = ctx.enter_context(tc.tile_pool(name="ps_c", bufs=2, space="PSUM"))
            gidx = ctx.enter_context(tc.tile_pool(name="gidx", bufs=3))
            gdst = ctx.enter_context(tc.tile_pool(name="gdst", bufs=2))
            fold = ctx.enter_context(tc.tile_pool(name="fold", bufs=2))
            fin = ctx.enter_context(tc.tile_pool(name="fin", bufs=2))

            y_aug = dram.tile([NK * CS, CH], BF16)

            ident = consts.tile([128, 128], F32)
            make_identity(nc, ident[:])
            wth_sb = consts.tile([CH, CH], F32)
            nc.sync.dma_start(out=wth_sb[:], in_=wth[:])
            wph_sb = consts.tile([CH, CH], F32)
            nc.sync.dma_start(out=wph_sb[:], in_=wph[:])
            y_own_sb = consts.tile([128, T * 128], F32)
            probe = consts.tile([128, CH], BF16)
            nc.gpsimd.memset(probe[:], 0.0)
            cst128 = consts.tile([128, CH], BF16)
            nc.gpsimd.memset(cst128[:], 1.0)
            sent = consts.tile([1, CH], BF16)
            nc.gpsimd.memset(sent[:], SENT_VAL)
            for k in range(NK):
                nc.sync.dma_start(out=y_aug[k * CS + CR: k * CS + CR + 1, :], in_=sent[:])

            A_MODE = os.environ.get("A_MODE", "full")
            PS_BUFS = int(os.environ.get("PS_BUFS", "2"))
            # ---------------- Phase A: y_aug = (x @ W_theta.T).bf16 ----------
            def emit_group(src, n0, gn, dst):
                """Process rows [n0, n0+gn) of src -> y into dst.
                dst = ("aug",) writes y_aug rows (with chunk-boundary split),
                dst = ("own",) writes y_own_sb cols."""
                nt = (gn + 127) // 128
                xg = ax.tile([128, nt * 128], F32, tag="xg",
                             bufs=int(os.environ.get("XG_BUFS", "2")))
                xg3 = xg[:].rearrange("p (i c) -> p i c", c=CH)
                load_eng = nc.gpsimd if A_MODE == "dma3" else nc.sync
                if gn % 128 == 0:
                    load_eng.dma_start(
                        out=xg3[:, :nt, :],
                        in_=src[n0: n0 + gn, :].rearrange("(i p) c -> p i c", p=128))
                else:
                    for i in range(nt):
                        rn = min(128, gn - i * 128)
                        nc.sync.dma_start(
                            out=xg3[:rn, i, :],
                            in_=src[n0 + i * 128: n0 + i * 128 + rn, :])
                if A_MODE.startswith("dma") and dst == "aug":
                    # dma : load->store dep, both on sync
                    # dma2: stores only dep-free (loads still emitted)
                    # dma3: load on gpsimd, stores dep on load, on sync
                    # dma5: loads only (no stores)
                    if gn % 128 == 0:
                        for i in range(nt):
                            r0 = n0 + i * 128
                            kb = r0 // CR
                            if A_MODE == "dma5":
                                continue
                            src_ap = (cst128[:] if A_MODE == "dma2"
                                      else xg3[:, i, :CH // 2].bitcast(BF16))
                            nc.sync.dma_start(
                                out=y_aug[r0 + kb: r0 + kb + 128, :], in_=src_ap)
                    return
                pt = ps_t.tile([128, nt * 128], F32, tag="pt", bufs=PS_BUFS)
                for i in range(nt):
                    rn = min(128, gn - i * 128)
                    nc.tensor.transpose(
                        out=pt[:, i * 128: i * 128 + rn],
                        in_=xg3[:rn, i, :],
                        identity=ident[:rn, :rn])
                xT = axT.tile([128, nt * 128], F32, tag="xT")
                nc.vector.tensor_copy(out=xT[:, : nt * 128], in_=pt[:, : nt * 128])
                if A_MODE == "nomm" and dst == "aug":
                    for i in range(nt):
                        r0 = n0 + i * 128
                        kb = r0 // CR
                        nc.sync.dma_start(
                            out=y_aug[r0 + kb: r0 + kb + 128, :],
                            in_=xT[:, i * 128: i * 128 + 128][:, :CH // 2].bitcast(BF16))
                    return
                py = ps_y.tile([128, nt * 128], F32, tag="py", bufs=PS_BUFS)
                for i in range(nt):
                    rn = min(128, gn - i * 128)
                    nc.tensor.matmul(
                        out=py[:rn, i * 128: (i + 1) * 128],
                        lhsT=xT[:, i * 128: i * 128 + rn],
                        rhs=wth_sb[:],
                        start=True, stop=True)
                if dst == "own":
                    nc.scalar.copy(
                        out=y_own_sb[:, n0: n0 + nt * 128], in_=py[:, : nt * 128])
                    return
                yg = ay.tile([128, nt * 128], BF16, tag="yg")
                copy2 = nc.vector.tensor_copy if A_MODE == "dvecopy" else nc.scalar.copy
                if gn % 128 == 0:
                    copy2(out=yg[:, : gn], in_=py[:, : gn])
                else:
                    for i in range(nt):
                        rn = min(128, gn - i * 128)
                        copy2(
                            out=yg[:rn, i * 128: (i + 1) * 128],
                            in_=py[:rn, i * 128: (i + 1) * 128])
                yg3 = yg[:].rearrange("p (i c) -> p i c", c=CH)
                # write y rows n -> aug rows n + n // CR, splitting at tile level
                for i in range(nt):
                    r0 = n0 + i * 128
                    rn = min(128, gn - i * 128)
                    kb = r0 // CR
                    ke = (r0 + rn - 1) // CR
                    if kb == ke:
                        nc.sync.dma_start(
                            out=y_aug[r0 + kb: r0 + kb + rn, :], in_=yg3[:rn, i, :])
                    else:
                        split = (kb + 1) * CR - r0       # rows before boundary
                        nc.sync.dma_start(
                            out=y_aug[r0 + kb: r0 + kb + split, :],
                            in_=yg3[:split, i, :])
                        nc.sync.dma_start(
                            out=y_aug[r0 + split + ke: r0 + ke + rn, :],
                            in_=yg3[split:rn, i, :])

            by_tile = {}
            for (t, k, g0, nb, coff) in insts:
                by_tile.setdefault(t, []).append((k, g0, nb, coff))

            for _rep in range(reps):
              for n0 in range(0, N, 512):
                emit_group(x_full, n0, min(512, N - n0), "aug")
              for n0 in range(0, T * 128, 512):
                emit_group(x_own, n0, min(512, T * 128 - n0), "own")

              # ---------------- Phase B + C per tile ---------------------------
              for t in range(T):
                 c0, c1 = tile_cols[t]
                 it = gidx.tile([128, max(c1 - c0, 16)], I16, tag="it")
                 if c1 > c0:
                     nc.sync.dma_start(out=it[:, : c1 - c0], in_=idxw[:, c0:c1])
                 dks = {}
                 for k in range(NK):
                     btk = int(B[t, k])
                     if btk == 0:
                         continue
                     dks[k] = gdst.tile([128, btk * CH], BF16, tag=f"g{k}", name=f"dk{k}")
                 for (k, g0, nb, coff) in by_tile.get(t, []):
                     dk3 = dks[k][:].rearrange("p (b c) -> p b c", c=CH)
                     n_i = nb * 128
                     nc.gpsimd.dma_gather(
                         out_ap=dk3[:, g0: g0 + nb, :],
                         in_ap=y_aug[k * CS: (k + 1) * CS, :],
                         idxs_ap=it[:, coff - c0: coff - c0 + n_i // 16],
                         num_idxs=n_i,
                         num_idxs_reg=n_i,
                         elem_size=CH,
                         queue_num=qc[0] % 4,
                     )
                     qc[0] += 1
                 # fold each chunk's rect down to one [128, CH] min
                 mks = []
                 for k in range(NK):
                     if k not in dks:
                         continue
                     cur = dks[k]
                     nb = int(B[t, k])
                     while nb > 1:
                         half = (nb + 1) // 2
                         nxt = fold.tile([128, half * CH], BF16, tag=f"f{k}", bufs=3)
                         nc.vector.tensor_tensor(
                             out=nxt[:, : half * CH],
                             in0=cur[:, : half * CH],
                             in1=cur[:, (nb - half) * CH: nb * CH],
                             op=mybir.AluOpType.min)
                         cur, nb = nxt, half
                     mks.append(cur)
                 m = fin.tile([128, CH], F32, tag="m")
                 if len(mks) == 0:
                     nc.gpsimd.memset(m[:], SENT_VAL)
                 elif len(mks) == 1:
                     nc.vector.tensor_copy(out=m[:], in_=mks[0][:, :CH])
                 else:
                     # sequential accumulate with alternating tags (max 2 live)
                     acc = mks[0]
                     for i in range(1, len(mks) - 1):
                         mm = fold.tile([128, CH], BF16, tag=f"mrg{i % 2}")
                         nc.vector.tensor_tensor(
                             out=mm[:], in0=acc[:, :CH], in1=mks[i][:, :CH],
                             op=mybir.AluOpType.min)
                         acc = mm
                     nc.vector.tensor_tensor(
                         out=m[:], in0=acc[:, :CH], in1=mks[-1][:, :CH],
                         op=mybir.AluOpType.min)
                 # aggr = y_own - m ; out_tile = aggr @ W_phi.T
                 aggr = fin.tile([128, CH], F32, tag="aggr")
                 nc.vector.tensor_sub(
                     out=aggr[:], in0=y_own_sb[:, t * 128: (t + 1) * 128], in1=m[:])
                 ptr = ps_c.tile([128, CH], F32, tag="ctr")
                 nc.tensor.transpose(out=ptr[:], in_=aggr[:], identity=ident[:])
                 aggrT = fin.tile([128, CH], F32, tag="aggrT")
                 nc.vector.tensor_copy(out=aggrT[:], in_=ptr[:])
                 po = ps_c.tile([128, CH], F32, tag="cmm")
                 nc.tensor.matmul(out=po[:], lhsT=aggrT[:], rhs=wph_sb[:],
                                  start=True, stop=True)
                 osb = fin.tile([128, CH], F32, tag="osb")
                 nc.scalar.copy(out=osb[:], in_=po[:])
                 nc.sync.dma_start(out=out[t * 128: (t + 1) * 128, :], in_=osb[:])

            if phases != "abc":
                fillz = consts.tile([128, CH], F32)
                nc.vector.tensor_copy(out=fillz[:], in_=probe[:])
                for t in range(T):
                    nc.sync.dma_start(out=out[t * 128: (t + 1) * 128, :], in_=fillz[:])
    nc.compile()
    return nc


_CACHE = {}


def _get_runner_and_plan(x, edge_index, cfg: Cfg, reps=1, phases="abc"):
    plan, idxw, x_own, own_nodes, deg = prep(x, edge_index, cfg)
    skey = (cfg, reps, phases, tuple(plan["B"].reshape(-1).tolist()))
    if skey not in _CACHE:
        nc = build_program(plan, reps=reps, phases=phases)
        _CACHE[skey] = BassRunner(nc, cfg.n_cores)
    return _CACHE[skey], plan, idxw, x_own, own_nodes, deg


def run_cfg(x, edge_index, W_theta, W_phi, cfg: Cfg, time_iters=0, reps=1, phases="abc"):
    runner, plan, idxw, x_own, own_nodes, deg = _get_runner_and_plan(x, edge_index, cfg, reps=reps, phases=phases)
    wtt = np.ascontiguousarray(np.asarray(W_theta, np.float32).T)
    wpt = np.ascontiguousarray(np.asarray(W_phi, np.float32).T)
    x_np = np.asarray(x, np.float32)
    in_maps = [
        dict(x_full=x_np, x_own=x_own[c], w_theta_t=wtt, w_phi_t=wpt,
             idxw=np.ascontiguousarray(idxw[c]) if plan["W_total"] > 0
             else np.zeros((128, 16), np.int16))
        for c in range(cfg.n_cores)
    ]
    runner.prepare(in_maps)
    outs = runner.run()
    t_ns = runner.time_ns(iters=time_iters) if time_iters else None
    res = runner.results(outs)
    out_full = np.empty((cfg.N, CH), np.float32)
    for c in range(cfg.n_cores):
        out_full[own_nodes[c]] = res[c]["out"][: cfg.npc]
    out_full[deg == 0] = 0.0
    return out_full, t_ns


def kernel(x, edge_index, W_theta, W_phi):
    out, _ = run_cfg(x, edge_index, W_theta, W_phi, Cfg())
    return out



# revision 8
# speedup vs baseline: 1.4684x; 1.4684x over previous
"""Trainium2 Bass kernel for DevConv-style GNN message passing.

Reference computation:
    rel_t = (x[row] - x[col]) @ W_theta.T          # [E, 128]
    aggr  = segment_max(rel_t, row, N)             # [N, 128], empty -> 0
    out   = aggr @ W_phi.T                         # [N, 128]

Key reformulation: with y = x @ W_theta.T, within a segment (fixed row d)
    max_e (y[d] - y[col_e]) = y[d] - min_e y[col_e]     (per channel)
so the per-edge matmul disappears and only ONE gather per edge (y[col]) is
needed, followed by a segmented min.

Distribution: nodes are assigned to the 8 cores by degree-rank striping
(rank r -> core r % 8). Each core:
  Phase A: computes y = x @ W_theta.T for ALL nodes (bf16) into an HBM
           table that is split into 4 chunks of <=32767 rows (+1 sentinel
           row of +3e38 per chunk) because dma_gather indices are int16.
  Phase B: for each 128-node tile and each chunk, gathers y[col] rows into
           a padded [128 nodes x B slots] SBUF rect via dma_gather
           (pad slots point at the chunk sentinel), then pairwise-min folds
           the slots and merges chunks -> m[d] = min_e y[col_e].
  Phase C: aggr = y_own - m  (y_own computed on-chip from x_own),
           out_tile = aggr @ W_phi.T via PE transpose + matmul.
Host un-permutes the concatenated core outputs and zeroes empty nodes.
"""
import sys
import os

sys.path.insert(0, "/opt/trn_rl_repo")

from contextlib import ExitStack
from dataclasses import dataclass

import numpy as np
import ml_dtypes

import concourse.bass as bass
import concourse.tile as tile
from concourse import bacc, mybir
from concourse.masks import make_identity

import time

import jax
from jax.sharding import Mesh, PartitionSpec
from jax.experimental.shard_map import shard_map

from concourse.bass2jax import (
    _bass_exec_p, install_neuronx_cc_hook, partition_id_tensor)


class BassRunner:
    """Keeps a jitted PJRT executable for a Bass program so it can be run
    repeatedly on device-resident inputs (for wall-clock timing)."""

    def __init__(self, nc, n_cores: int):
        install_neuronx_cc_hook()
        self.nc = nc
        self.n_cores = n_cores
        partition_name = nc.partition_id_tensor.name if nc.partition_id_tensor else None
        in_names, out_names, out_avals = [], [], []
        for alloc in nc.m.functions[0].allocations:
            if not isinstance(alloc, mybir.MemoryLocationSet):
                continue
            name = alloc.memorylocations[0].name
            if alloc.kind == "ExternalInput":
                if name != partition_name:
                    in_names.append(name)
            elif alloc.kind == "ExternalOutput":
                out_names.append(name)
                out_avals.append(jax.core.ShapedArray(
                    tuple(alloc.tensor_shape), mybir.dt.np(alloc.dtype)))
        self.in_names, self.out_names, self.out_avals = in_names, out_names, out_avals
        self.n_params = len(in_names)
        all_in_names = list(in_names) + list(out_names)
        if partition_name is not None:
            all_in_names.append(partition_name)

        def _body(*args):
            operands = list(args)
            if partition_name is not None:
                operands.append(partition_id_tensor())
            outs = _bass_exec_p.bind(
                *operands,
                out_avals=tuple(out_avals),
                in_names=tuple(all_in_names),
                out_names=tuple(out_names),
                lowering_input_output_aliases=(),
                sim_require_finite=True,
                sim_require_nnan=True,
                nc=nc,
            )
            return tuple(outs)

        devices = jax.devices()[:n_cores]
        self.mesh = Mesh(np.asarray(devices), ("core",))
        n_outs = len(out_names)
        in_specs = (PartitionSpec("core"),) * (self.n_params + n_outs)
        out_specs = (PartitionSpec("core"),) * n_outs
        self.fn = jax.jit(
            shard_map(_body, mesh=self.mesh, in_specs=in_specs,
                      out_specs=out_specs, check_rep=False),
            keep_unused=True,
        )
        self._dev_args = None

    def prepare(self, in_maps):
        assert len(in_maps) == self.n_cores
        concat_in = [
            np.concatenate([np.asarray(in_maps[c][n]) for c in range(self.n_cores)],
                           axis=0)
            for n in self.in_names
        ]
        concat_zeros = [
            np.zeros((self.n_cores * a.shape[0], *a.shape[1:]), a.dtype)
            for a in self.out_avals
        ]
        sharding = jax.sharding.NamedSharding(self.mesh, PartitionSpec("core"))
        self._dev_args = [jax.device_put(v, sharding) for v in concat_in + concat_zeros]
        return self

    def run(self):
        outs = self.fn(*self._dev_args)
        jax.block_until_ready(outs)
        return outs

    def results(self, outs):
        return [
            {n: np.asarray(outs[i]).reshape(self.n_cores, *self.out_avals[i].shape)[c]
             for i, n in enumerate(self.out_names)}
            for c in range(self.n_cores)
        ]

    def time_ns(self, iters=5, warmup=2):
        for _ in range(warmup):
            self.run()
        ts = []
        for _ in range(iters):
            t0 = time.perf_counter()
            self.run()
            ts.append((time.perf_counter() - t0) * 1e9)
        return min(ts)


CH = 128
F32 = mybir.dt.float32
BF16 = mybir.dt.bfloat16
I16 = mybir.dt.int16
SENT_VAL = 3.0e38
IDX_PER_INST = 1024          # dma_gather crashes >= 2048 idx/instruction


@dataclass(frozen=True)
class Cfg:
    N: int = 100_000
    E: int = 3_200_000
    n_cores: int = 8
    chunk_real: int = 32_767     # rows addressable by int16 (sentinel at chunk_real)

    @property
    def n_chunks(self):
        return (self.N + self.chunk_real - 1) // self.chunk_real

    @property
    def chunk_stride(self):
        return self.chunk_real + 1

    @property
    def npc(self):
        assert self.N % self.n_cores == 0
        return self.N // self.n_cores

    @property
    def tiles(self):
        return (self.npc + 127) // 128


def _wrap16(seg: np.ndarray) -> np.ndarray:
    """Per-instruction idx wrap: flat [n] -> [128, n//16]; idx i at
    (partition i%16, col i//16), replicated across the 8 gpsimd groups."""
    n = seg.shape[-1]
    w = seg.reshape(*seg.shape[:-1], n // 16, 16)
    w = np.swapaxes(w, -1, -2)                       # [..., 16, n//16]
    return np.tile(w, (1,) * (seg.ndim - 1) + (8, 1))


def prep(x, edge_index, cfg: Cfg, ipi: int = IDX_PER_INST):
    """Host-side data prep. Returns (plan, per-core inputs, unpermute info)."""
    BLK_PER_INST = ipi // 128
    N, E, NC = cfg.N, cfg.E, cfg.n_cores
    CR, NK, T = cfg.chunk_real, cfg.n_chunks, cfg.tiles
    row = np.asarray(edge_index[0], dtype=np.int64)
    col = np.asarray(edge_index[1], dtype=np.int64)

    deg = np.bincount(row, minlength=N)
    order = np.argsort(-deg, kind="stable")          # node ids by desc degree
    core_of = np.empty(N, np.int64)
    pos_of = np.empty(N, np.int64)
    r = np.arange(N)
    core_of[order] = r % NC
    pos_of[order] = r // NC

    ec = core_of[row]
    ep = pos_of[row]
    ek = col // CR
    elocal = (col - ek * CR).astype(np.int16)
    et = ep // 128
    ed = ep % 128

    # per-(core,tile,chunk,node) counts and within-group slot index j
    key = ((ec * T + et) * NK + ek) * 128 + ed
    o = np.argsort(key, kind="stable")
    ks = key[o]
    first = np.r_[True, ks[1:] != ks[:-1]]
    run_id = np.cumsum(first) - 1
    run_start = np.flatnonzero(first)
    j = np.arange(E) - run_start[run_id]

    cnt = np.bincount(key, minlength=NC * T * NK * 128).reshape(NC, T, NK, 128)
    B = cnt.max(axis=(0, 3)).astype(np.int64)        # [T, NK] shared structure

    Bf = B.reshape(-1)
    off = np.concatenate([[0], np.cumsum(Bf * 128)])  # slot offset per (t,k)
    total_slots = int(off[-1])

    idx_all = np.full((NC, total_slots), CR, np.int16)   # sentinel local idx
    tk = et[o] * NK + ek[o]
    pos_in = off[tk] + j * 128 + ed[o]
    idx_all[ec[o], pos_in] = elocal[o]

    # split into gather instructions and build wrapped idx input
    insts = []           # (t, k, g0blk, nblk, col_off)
    tile_cols = []       # per tile: (col_start, col_end)
    wsegs = []
    col_off = 0
    for t in range(T):
        t_start = col_off
        for k in range(NK):
            btk = int(B[t, k])
            base = int(off[t * NK + k])
            for g0 in range(0, btk, BLK_PER_INST):
                nb = min(BLK_PER_INST, btk - g0)
                n_i = nb * 128
                seg = idx_all[:, base + g0 * 128: base + g0 * 128 + n_i]
                wsegs.append(_wrap16(seg))
                insts.append((t, k, g0, nb, col_off))
                col_off += n_i // 16
        tile_cols.append((t_start, col_off))
    idxw = np.concatenate(wsegs, axis=2) if wsegs else np.zeros((NC, 128, 0), np.int16)
    W_total = idxw.shape[2]

    # per-core x_own in pos order, padded to T*128 rows
    own_nodes = np.empty((NC, cfg.npc), np.int64)
    own_nodes[core_of[order], pos_of[order]] = order  # own_nodes[c, p] = node id
    x_np = np.asarray(x, dtype=np.float32)
    x_own = np.zeros((NC, T * 128, CH), np.float32)
    x_own[:, : cfg.npc] = x_np[own_nodes]

    plan = dict(cfg=cfg, B=B, insts=insts, tile_cols=tile_cols, W_total=W_total)
    return plan, idxw, x_own, own_nodes, deg


def build_program(plan, reps=1, phases="agfc", queues=4):
    cfg: Cfg = plan["cfg"]
    N, NK, CR, T = cfg.N, cfg.n_chunks, cfg.chunk_real, cfg.tiles
    CS = cfg.chunk_stride
    B, insts, tile_cols, W_total = (
        plan["B"], plan["insts"], plan["tile_cols"], plan["W_total"])

    nc = bacc.Bacc(None, target_bir_lowering=False, num_swdge_queues=queues)
    x_full = nc.declare_dram_parameter("x_full", [N, CH], F32, isOutput=False)
    x_own = nc.declare_dram_parameter("x_own", [T * 128, CH], F32, isOutput=False)
    wth = nc.declare_dram_parameter("w_theta_t", [CH, CH], F32, isOutput=False)
    wph = nc.declare_dram_parameter("w_phi_t", [CH, CH], F32, isOutput=False)
    idxw = nc.declare_dram_parameter("idxw", [128, max(W_total, 16)], I16, isOutput=False)
    out = nc.declare_dram_parameter("out", [T * 128, CH], F32, isOutput=True)

    qc = [0]  # gather queue rotation

    with tile.TileContext(nc) as tc:
        with ExitStack() as ctx:
            consts = ctx.enter_context(tc.tile_pool(name="consts", bufs=1))
            dram = ctx.enter_context(tc.tile_pool(name="dram", bufs=1, space="DRAM"))
            ax = ctx.enter_context(tc.tile_pool(name="ax", bufs=2))
            axT = ctx.enter_context(tc.tile_pool(name="axT", bufs=2))
            ay = ctx.enter_context(tc.tile_pool(name="ay", bufs=2))
            ps_t = ctx.enter_context(tc.tile_pool(name="ps_t", bufs=2, space="PSUM"))
            ps_y = ctx.enter_context(tc.tile_pool(name="ps_y", bufs=2, space="PSUM"))
            ps_c = ctx.enter_context(tc.tile_pool(name="ps_c", bufs=2, space="PSUM"))
            gidx = ctx.enter_context(tc.tile_pool(name="gidx", bufs=3))
            gdst = ctx.enter_context(tc.tile_pool(name="gdst", bufs=2))
            fold = ctx.enter_context(tc.tile_pool(name="fold", bufs=2))
            fin = ctx.enter_context(tc.tile_pool(name="fin", bufs=2))

            y_aug = dram.tile([NK * CS, CH], BF16)

            ident = consts.tile([128, 128], F32)
            make_identity(nc, ident[:])
            wth_sb = consts.tile([CH, CH], F32)
            nc.sync.dma_start(out=wth_sb[:], in_=wth[:])
            wph_sb = consts.tile([CH, CH], F32)
            nc.sync.dma_start(out=wph_sb[:], in_=wph[:])
            y_own_sb = consts.tile([128, T * 128], F32)
            probe = consts.tile([128, CH], BF16)
            nc.gpsimd.memset(probe[:], 0.0)
            cst128 = consts.tile([128, CH], BF16)
            nc.gpsimd.memset(cst128[:], 1.0)
            sent = consts.tile([1, CH], BF16)
            nc.gpsimd.memset(sent[:], SENT_VAL)
            for k in range(NK):
                nc.sync.dma_start(out=y_aug[k * CS + CR: k * CS + CR + 1, :], in_=sent[:])

            A_MODE = os.environ.get("A_MODE", "full")
            PS_BUFS = int(os.environ.get("PS_BUFS", "2"))
            # ---------------- Phase A: y_aug = (x @ W_theta.T).bf16 ----------
            def emit_group(src, n0, gn, dst):
                """Process rows [n0, n0+gn) of src -> y into dst.
                dst = ("aug",) writes y_aug rows (with chunk-boundary split),
                dst = ("own",) writes y_own_sb cols."""
                nt = (gn + 127) // 128
                xg = ax.tile([128, nt * 128], F32, tag="xg",
                             bufs=int(os.environ.get("XG_BUFS", "2")))
                xg3 = xg[:].rearrange("p (i c) -> p i c", c=CH)
                load_eng = nc.gpsimd if A_MODE == "dma3" else nc.sync
                if gn % 128 == 0:
                    load_eng.dma_start(
                        out=xg3[:, :nt, :],
                        in_=src[n0: n0 + gn, :].rearrange("(i p) c -> p i c", p=128))
                else:
                    for i in range(nt):
                        rn = min(128, gn - i * 128)
                        nc.sync.dma_start(
                            out=xg3[:rn, i, :],
                            in_=src[n0 + i * 128: n0 + i * 128 + rn, :])
                if A_MODE.startswith("dma") and dst == "aug":
                    # dma : load->store dep, both on sync
                    # dma2: stores only dep-free (loads still emitted)
                    # dma3: load on gpsimd, stores dep on load, on sync
                    # dma5: loads only (no stores)
                    if gn % 128 == 0:
                        for i in range(nt):
                            r0 = n0 + i * 128
                            kb = r0 // CR
                            if A_MODE == "dma5":
                                continue
                            src_ap = (cst128[:] if A_MODE == "dma2"
                                      else xg3[:, i, :CH // 2].bitcast(BF16))
                            nc.sync.dma_start(
                                out=y_aug[r0 + kb: r0 + kb + 128, :], in_=src_ap)
                    return
                pt = ps_t.tile([128, nt * 128], F32, tag="pt", bufs=PS_BUFS)
                for i in range(nt):
                    rn = min(128, gn - i * 128)
                    nc.tensor.transpose(
                        out=pt[:, i * 128: i * 128 + rn],
                        in_=xg3[:rn, i, :],
                        identity=ident[:rn, :rn])
                xT = axT.tile([128, nt * 128], F32, tag="xT")
                nc.vector.tensor_copy(out=xT[:, : nt * 128], in_=pt[:, : nt * 128])
                if A_MODE == "nomm" and dst == "aug":
                    for i in range(nt):
                        r0 = n0 + i * 128
                        kb = r0 // CR
                        nc.sync.dma_start(
                            out=y_aug[r0 + kb: r0 + kb + 128, :],
                            in_=xT[:, i * 128: i * 128 + 128][:, :CH // 2].bitcast(BF16))
                    return
                py = ps_y.tile([128, nt * 128], F32, tag="py", bufs=PS_BUFS)
                for i in range(nt):
                    rn = min(128, gn - i * 128)
                    nc.tensor.matmul(
                        out=py[:rn, i * 128: (i + 1) * 128],
                        lhsT=xT[:, i * 128: i * 128 + rn],
                        rhs=wth_sb[:],
                        start=True, stop=True)
                if dst == "own":
                    nc.scalar.copy(
                        out=y_own_sb[:, n0: n0 + nt * 128], in_=py[:, : nt * 128])
                    return
                yg = ay.tile([128, nt * 128], BF16, tag="yg")
                copy2 = nc.vector.tensor_copy if A_MODE == "dvecopy" else nc.scalar.copy
                if gn % 128 == 0:
                    copy2(out=yg[:, : gn], in_=py[:, : gn])
                else:
                    for i in range(nt):
                        rn = min(128, gn - i * 128)
                        copy2(
                            out=yg[:rn, i * 128: (i + 1) * 128],
                            in_=py[:rn, i * 128: (i + 1) * 128])
                yg3 = yg[:].rearrange("p (i c) -> p i c", c=CH)
                # write y rows n -> aug rows n + n // CR, splitting at tile level
                for i in range(nt):
                    r0 = n0 + i * 128
                    rn = min(128, gn - i * 128)
                    kb = r0 // CR
                    ke = (r0 + rn - 1) // CR
                    if kb == ke:
                        nc.sync.dma_start(
                            out=y_aug[r0 + kb: r0 + kb + rn, :], in_=yg3[:rn, i, :])
                    else:
                        split = (kb + 1) * CR - r0       # rows before boundary
                        nc.sync.dma_start(
                            out=y_aug[r0 + kb: r0 + kb + split, :],
                            in_=yg3[:split, i, :])
                        nc.sync.dma_start(
                            out=y_aug[r0 + split + ke: r0 + ke + rn, :],
                            in_=yg3[split:rn, i, :])

            by_tile = {}
            for (t, k, g0, nb, coff) in insts:
                by_tile.setdefault(t, []).append((k, g0, nb, coff))

            for _rep in range(reps):
              if "a" in phases:
                with tc.spectator_scope("phaseA"):
                  for n0 in range(0, N, 512):
                    emit_group(x_full, n0, min(512, N - n0), "aug")
                  for n0 in range(0, T * 128, 512):
                    emit_group(x_own, n0, min(512, T * 128 - n0), "own")

              # ---------------- Phase B + C per tile ---------------------------
              for t in range(T):
                 c0, c1 = tile_cols[t]
                 dks = {}
                 if "g" in phases:
                   with tc.spectator_scope("phb_idx"):
                     it = gidx.tile([128, max(c1 - c0, 16)], I16, tag="it")
                     if c1 > c0:
                         nc.sync.dma_start(out=it[:, : c1 - c0], in_=idxw[:, c0:c1])
                   for k in range(NK):
                     btk = int(B[t, k])
                     if btk == 0:
                         continue
                     dks[k] = gdst.tile([128, btk * CH], BF16, tag=f"g{k}", name=f"dk{k}")
                   with tc.spectator_scope("phb_gather"):
                    for (k, g0, nb, coff) in by_tile.get(t, []):
                     dk3 = dks[k][:].rearrange("p (b c) -> p b c", c=CH)
                     n_i = nb * 128
                     nc.gpsimd.dma_gather(
                         out_ap=dk3[:, g0: g0 + nb, :],
                         in_ap=y_aug[k * CS: (k + 1) * CS, :],
                         idxs_ap=it[:, coff - c0: coff - c0 + n_i // 16],
                         num_idxs=n_i,
                         num_idxs_reg=n_i,
                         elem_size=CH,
                         queue_num=qc[0] % queues,
                     )
                     qc[0] += 1
                 if "f" not in phases:
                     continue
                 # fold each chunk's rect down to one [128, CH] min
                 mks = []
                 with tc.spectator_scope("phb_fold"):
                  for k in range(NK):
                     if k not in dks:
                         continue
                     cur = dks[k]
                     nb = int(B[t, k])
                     while nb > 1:
                         half = (nb + 1) // 2
                         nxt = fold.tile([128, half * CH], BF16, tag=f"f{k}", bufs=3)
                         nc.vector.tensor_tensor(
                             out=nxt[:, : half * CH],
                             in0=cur[:, : half * CH],
                             in1=cur[:, (nb - half) * CH: nb * CH],
                             op=mybir.AluOpType.min)
                         cur, nb = nxt, half
                     mks.append(cur)
                 with tc.spectator_scope("phb_merge"):
                  m = fin.tile([128, CH], F32, tag="m")
                  if len(mks) == 0:
                     nc.gpsimd.memset(m[:], SENT_VAL)
                  elif len(mks) == 1:
                     nc.vector.tensor_copy(out=m[:], in_=mks[0][:, :CH])
                  else:
                     # sequential accumulate with alternating tags (max 2 live)
                     acc = mks[0]
                     for i in range(1, len(mks) - 1):
                         mm = fold.tile([128, CH], BF16, tag=f"mrg{i % 2}")
                         nc.vector.tensor_tensor(
                             out=mm[:], in0=acc[:, :CH], in1=mks[i][:, :CH],
                             op=mybir.AluOpType.min)
                         acc = mm
                     nc.vector.tensor_tensor(
                         out=m[:], in0=acc[:, :CH], in1=mks[-1][:, :CH],
                         op=mybir.AluOpType.min)
                 if "c" not in phases:
                     continue
                 # aggr = y_own - m ; out_tile = aggr @ W_phi.T
                 with tc.spectator_scope("phaseC"):
                  aggr = fin.tile([128, CH], F32, tag="aggr")
                  nc.vector.tensor_sub(
                     out=aggr[:], in0=y_own_sb[:, t * 128: (t + 1) * 128], in1=m[:])
                  ptr = ps_c.tile([128, CH], F32, tag="ctr")
                  nc.tensor.transpose(out=ptr[:], in_=aggr[:], identity=ident[:])
                  aggrT = fin.tile([128, CH], F32, tag="aggrT")
                  nc.vector.tensor_copy(out=aggrT[:], in_=ptr[:])
                  po = ps_c.tile([128, CH], F32, tag="cmm")
                  nc.tensor.matmul(out=po[:], lhsT=aggrT[:], rhs=wph_sb[:],
                                  start=True, stop=True)
                  osb = fin.tile([128, CH], F32, tag="osb")
                  nc.scalar.copy(out=osb[:], in_=po[:])
                  nc.sync.dma_start(out=out[t * 128: (t + 1) * 128, :], in_=osb[:])

            if "c" not in phases:
                fillz = consts.tile([128, CH], F32)
                nc.vector.tensor_copy(out=fillz[:], in_=probe[:])
                for t in range(T):
                    nc.sync.dma_start(out=out[t * 128: (t + 1) * 128, :], in_=fillz[:])
    nc.compile()
    return nc


_CACHE = {}


def _get_runner_and_plan(x, edge_index, cfg: Cfg, reps=1, phases="agfc",
                         queues=4, ipi=IDX_PER_INST):
    plan, idxw, x_own, own_nodes, deg = prep(x, edge_index, cfg, ipi=ipi)
    skey = (cfg, reps, phases, queues, ipi,
            tuple(plan["B"].reshape(-1).tolist()))
    if skey not in _CACHE:
        nc = build_program(plan, reps=reps, phases=phases, queues=queues)
        _CACHE[skey] = BassRunner(nc, cfg.n_cores)
    return _CACHE[skey], plan, idxw, x_own, own_nodes, deg


def run_cfg(x, edge_index, W_theta, W_phi, cfg: Cfg, time_iters=0, reps=1,
            phases="agfc", queues=4, ipi=IDX_PER_INST, idx_override=None):
    runner, plan, idxw, x_own, own_nodes, deg = _get_runner_and_plan(
        x, edge_index, cfg, reps=reps, phases=phases, queues=queues, ipi=ipi)
    if idx_override is not None:
        idxw = idx_override(idxw)
    wtt = np.ascontiguousarray(np.asarray(W_theta, np.float32).T)
    wpt = np.ascontiguousarray(np.asarray(W_phi, np.float32).T)
    x_np = np.asarray(x, np.float32)
    in_maps = [
        dict(x_full=x_np, x_own=x_own[c], w_theta_t=wtt, w_phi_t=wpt,
             idxw=np.ascontiguousarray(idxw[c]) if plan["W_total"] > 0
             else np.zeros((128, 16), np.int16))
        for c in range(cfg.n_cores)
    ]
    runner.prepare(in_maps)
    outs = runner.run()
    t_ns = runner.time_ns(iters=time_iters) if time_iters else None
    res = runner.results(outs)
    out_full = np.empty((cfg.N, CH), np.float32)
    for c in range(cfg.n_cores):
        out_full[own_nodes[c]] = res[c]["out"][: cfg.npc]
    out_full[deg == 0] = 0.0
    return out_full, t_ns


def kernel(x, edge_index, W_theta, W_phi):
    out, _ = run_cfg(x, edge_index, W_theta, W_phi, Cfg())
    return out



# revision 9
# speedup vs baseline: 5.6866x; 3.8725x over previous
"""Trainium2 Bass kernel for DevConv-style GNN message passing (final).

Reference computation:
    rel_t = (x[row] - x[col]) @ W_theta.T          # [E, 128]
    aggr  = segment_max(rel_t, row, N)             # [N, 128], empty -> 0
    out   = aggr @ W_phi.T                         # [N, 128]

Reformulation: with y = x @ W_theta.T, per destination node d
    max_e (y[d] - y[col_e]) = y[d] - min_e y[col_e]     (per channel)
    out[d] = x[d] @ (W_phi W_theta).T - m[d] @ W_phi.T  (m = the min)

Design (HW-measurement driven):
  - Plain dma_gather (HBM source, <=1024 idx/instruction, 4 swdge
    queues) is nearly free when pipelined; everything else is arranged
    to minimize per-instruction latency chains.
  - Phase A: host supplies xT permuted into K=8 chunks of 12543 nodes
    (+1 sentinel row); y = matmul(lhsT=xT_block, W_theta.T) -> PSUM ->
    ACT copy -> one big DMA per chunk into the HBM y table
    (partition-major token ids so the DMA is 128 large descriptors).
  - Gather: per (tile, chunk) into a shared per-tile rect of
    4-chunk-uniform depth; int16 idx = remapped local token ids; pads
    point at the sentinel row (value 1000.0).
  - Segmented min: in-place pairwise fold tree on the gather dst
    (contiguous tensor_tensor min), one merge into the bf16 acc.
  - Phase C per 4 tiles: PE-transpose m, then two accumulating matmuls
    (x_own @ Wc.T - m @ W_phi.T) -> one output DMA (partition-major).
  - Host: balanced chunk assignment (exponential-penalty greedy) +
    striping nodes across cores/tiles by worst per-chunk count, so the
    gather rect padding stays small and uniform.
Distribution: nodes striped across the 8 cores; each core owns its
destination nodes fully (edge-parallel by destination) - no cross-core
reduction needed. Weights replicated.
"""
import sys

sys.path.insert(0, "/opt/trn_rl_repo")

import time
from contextlib import ExitStack
from dataclasses import dataclass

import numpy as np
import ml_dtypes

import concourse.bass as bass
import concourse.tile as tile
from concourse import bacc, mybir
from concourse.masks import make_identity

import jax
from jax.sharding import Mesh, PartitionSpec
from jax.experimental.shard_map import shard_map

from concourse.bass2jax import (
    _bass_exec_p, install_neuronx_cc_hook, partition_id_tensor)


class BassRunner:
    """Keeps a jitted PJRT executable for a Bass program so it can be run
    repeatedly on device-resident inputs (for wall-clock timing)."""

    def __init__(self, nc, n_cores: int):
        install_neuronx_cc_hook()
        self.nc = nc
        self.n_cores = n_cores
        partition_name = nc.partition_id_tensor.name if nc.partition_id_tensor else None
        in_names, out_names, out_avals = [], [], []
        for alloc in nc.m.functions[0].allocations:
            if not isinstance(alloc, mybir.MemoryLocationSet):
                continue
            name = alloc.memorylocations[0].name
            if alloc.kind == "ExternalInput":
                if name != partition_name:
                    in_names.append(name)
            elif alloc.kind == "ExternalOutput":
                out_names.append(name)
                out_avals.append(jax.core.ShapedArray(
                    tuple(alloc.tensor_shape), mybir.dt.np(alloc.dtype)))
        self.in_names, self.out_names, self.out_avals = in_names, out_names, out_avals
        self.n_params = len(in_names)
        all_in_names = list(in_names) + list(out_names)
        if partition_name is not None:
            all_in_names.append(partition_name)

        def _body(*args):
            operands = list(args)
            if partition_name is not None:
                operands.append(partition_id_tensor())
            outs = _bass_exec_p.bind(
                *operands,
                out_avals=tuple(out_avals),
                in_names=tuple(all_in_names),
                out_names=tuple(out_names),
                lowering_input_output_aliases=(),
                sim_require_finite=True,
                sim_require_nnan=True,
                nc=nc,
            )
            return tuple(outs)

        devices = jax.devices()[:n_cores]
        self.mesh = Mesh(np.asarray(devices), ("core",))
        n_outs = len(out_names)
        in_specs = (PartitionSpec("core"),) * (self.n_params + n_outs)
        out_specs = (PartitionSpec("core"),) * n_outs
        self.fn = jax.jit(
            shard_map(_body, mesh=self.mesh, in_specs=in_specs,
                      out_specs=out_specs, check_rep=False),
            keep_unused=True,
        )
        self._dev_args = None

    def prepare(self, in_maps):
        assert len(in_maps) == self.n_cores
        concat_in = [
            np.concatenate([np.asarray(in_maps[c][n]) for c in range(self.n_cores)],
                           axis=0)
            for n in self.in_names
        ]
        concat_zeros = [
            np.zeros((self.n_cores * a.shape[0], *a.shape[1:]), a.dtype)
            for a in self.out_avals
        ]
        sharding = jax.sharding.NamedSharding(self.mesh, PartitionSpec("core"))
        self._dev_args = [jax.device_put(v, sharding) for v in concat_in + concat_zeros]
        return self

    def run(self):
        outs = self.fn(*self._dev_args)
        jax.block_until_ready(outs)
        return outs

    def results(self, outs):
        return [
            {n: np.asarray(outs[i]).reshape(self.n_cores, *self.out_avals[i].shape)[c]
             for i, n in enumerate(self.out_names)}
            for c in range(self.n_cores)
        ]

    def time_ns(self, iters=5, warmup=2):
        for _ in range(warmup):
            self.run()
        ts = []
        for _ in range(iters):
            t0 = time.perf_counter()
            self.run()
            ts.append((time.perf_counter() - t0) * 1e9)
        return min(ts)


CH = 128
F32 = mybir.dt.float32
BF16 = mybir.dt.bfloat16
I16 = mybir.dt.int16
SENT = 1000.0            # > any |y| value (y ~ N(0,1) scale)


@dataclass(frozen=True)
class Cfg:
    N: int = 100_000
    E: int = 3_200_000
    n_cores: int = 8
    K: int = 8               # node chunks
    CAPR: int = 12_543       # real tokens per chunk (sentinel at CAPR)
    HK: int = 2              # chunk halves (dst accumulation groups)

    @property
    def CS(self):
        return self.CAPR + 1  # 12544 = 98 ranks x 128

    @property
    def npc(self):
        return self.N // self.n_cores

    @property
    def T(self):
        return (self.npc + 127) // 128  # 98

    @property
    def KH(self):
        return self.K // self.HK        # chunks per half = 4


def _wrap16(seg: np.ndarray) -> np.ndarray:
    """Per-instruction idx wrap: flat [n] -> [128, n//16]; idx i at
    (partition i%16, col i//16), replicated across the 8 gpsimd groups."""
    n = seg.shape[-1]
    w = seg.reshape(*seg.shape[:-1], n // 16, 16)
    w = np.swapaxes(w, -1, -2)                       # [..., 16, n//16]
    return np.tile(w, (1,) * (seg.ndim - 1) + (8, 1))


def _balanced_chunks(row, col, N, K, CAPR, batch=32, refine_batch=64, W=8.0):
    """Assign each node (as col) to a chunk so each destination row's
    per-chunk neighbor counts stay uniform. Exponential penalty W**cnt
    (normalized per row by W**(deg/K)) targets the per-row max; one
    refinement pass after the greedy. Returns chunk_of[N], local_of[N]."""
    o = np.argsort(col, kind="stable")
    rs = np.asarray(row, np.int64)[o]
    coldeg = np.bincount(col, minlength=N)
    colptr = np.concatenate([[0], np.cumsum(coldeg)])
    proc = np.argsort(-coldeg, kind="stable")

    cnt = np.zeros((N, K), np.int32)
    capk = np.zeros(K, np.int64)
    chunk_of = np.full(N, -1, np.int64)
    rowdeg = np.bincount(np.asarray(row, np.int64), minlength=N)
    wrow = W ** (-rowdeg.astype(np.float64) / K)   # per-row normalizer

    def assign_batch(cs, removing):
        lens = coldeg[cs]
        tot = int(lens.sum())
        if tot:
            idx = np.concatenate([rs[colptr[c]: colptr[c + 1]] for c in cs])
            seg = np.repeat(np.arange(len(cs)), lens)
        else:
            idx = np.zeros(0, np.int64)
            seg = np.zeros(0, np.int64)
        if removing:
            ks0 = chunk_of[cs]
            if tot:
                np.subtract.at(cnt, (idx, np.repeat(ks0, lens)), 1)
            np.subtract.at(capk, ks0, 1)
        sc = np.zeros((len(cs), K), np.float64)
        if tot:
            np.add.at(sc, seg, wrow[idx, None] * W ** cnt[idx])
        sc += (capk >= CAPR)[None, :] * 1e30
        sc += capk[None, :] * 1e-6
        ks = np.argmin(sc, axis=1)
        over = capk[ks] >= CAPR
        if over.any():
            ks[over] = int(np.argmin(capk))
        chunk_of[cs] = ks
        np.add.at(capk, ks, 1)
        if tot:
            np.add.at(cnt, (idx, np.repeat(ks, lens)), 1)

    for b0 in range(0, N, batch):
        assign_batch(proc[b0: b0 + batch], removing=False)
    for b0 in range(0, N, refine_batch):
        assign_batch(proc[b0: b0 + refine_batch], removing=True)

    local_of = np.zeros(N, np.int64)
    fill = np.zeros(K, np.int64)
    for c in proc:
        k = chunk_of[c]
        local_of[c] = fill[k]
        fill[k] += 1
    return chunk_of, local_of


def prep(x, edge_index, cfg: Cfg):
    N, E, NC, K = cfg.N, cfg.E, cfg.n_cores, cfg.K
    CAPR, CS, T, KH = cfg.CAPR, cfg.CS, cfg.T, cfg.KH
    row = np.asarray(edge_index[0], dtype=np.int64)
    col = np.asarray(edge_index[1], dtype=np.int64)

    deg = np.bincount(row, minlength=N)

    chunk_of, local_of = _balanced_chunks(row, col, N, K, CAPR)

    # stripe nodes across cores/tiles by worst per-chunk count m
    percnt = np.zeros((N, K), np.int64)
    np.add.at(percnt, (row, chunk_of[col]), 1)
    m_of = percnt.max(axis=1)
    order = np.lexsort((-deg, -m_of))
    core_of = np.empty(N, np.int64)
    pos_of = np.empty(N, np.int64)
    r = np.arange(N)
    core_of[order] = r % NC
    pos_of[order] = r // NC

    ec = core_of[row]
    ep = pos_of[row]
    ek = chunk_of[col]
    et = ep // 128                                   # tile
    ed = ep % 128                                    # node in tile

    key = ((ec * T + et) * K + ek) * 128 + ed
    o = np.argsort(key, kind="stable")
    ks = key[o]
    first = np.r_[True, ks[1:] != ks[:-1]]
    run_start = np.flatnonzero(first)
    j = np.arange(E) - run_start[np.cumsum(first) - 1]

    cnt = np.bincount(key, minlength=NC * T * K * 128).reshape(NC, T, K, 128)
    Btk = cnt.max(axis=(0, 3)).astype(np.int64)      # [T, K]
    Bh = np.stack([Btk[:, h * KH:(h + 1) * KH].max(axis=1)
                   for h in range(cfg.HK)], axis=1)  # [T, HK]

    # idx arrays per chunk k: concat tiles t, each seg 128*Bh[t, h(k)]
    seg_off = np.zeros((T, K), np.int64)
    chunk_w = np.zeros(K, np.int64)
    for k in range(K):
        h = k // KH
        off = 0
        for t in range(T):
            seg_off[t, k] = off
            off += 128 * Bh[t, h]
        chunk_w[k] = off
    idx_all = [np.full((NC, int(chunk_w[k])), CAPR, np.int16) for k in range(K)]
    # slot-major positions within each (t,k) segment: pos = j*128 + d
    pos_in = seg_off[et[o], ek[o]] + j * 128 + ed[o]
    eco, eko = ec[o], ek[o]
    # y_hbm tokens are partition-major: local i -> row (i%128)*RANKS + i//128
    RANKS = CS // 128
    elo = local_of[col][o]
    elo = (elo % 128) * RANKS + elo // 128
    for k in range(K):
        m = eko == k
        idx_all[k][eco[m], pos_in[m]] = elo[m].astype(np.int16)
    idxw = [_wrap16(a) for a in idx_all]             # [K][NC, 128, w/16]

    # x_permT: [128, K*CS] bf16, col k*CS+i = x[node with chunk k local i].T
    x_np = np.asarray(x, np.float32)
    xpt = np.zeros((CH, K * CS), np.float32)
    xpt[:, chunk_of * CS + local_of] = x_np.T
    xpt = xpt.astype(ml_dtypes.bfloat16)

    # x_ownT per core: [128, T*128] bf16
    own_nodes = np.empty((NC, cfg.npc), np.int64)
    own_nodes[core_of[order], pos_of[order]] = order
    xot = np.zeros((NC, CH, T * 128), np.float32)
    for c in range(NC):
        xot[c, :, : cfg.npc] = x_np[own_nodes[c]].T
    xot = xot.astype(ml_dtypes.bfloat16)

    plan = dict(cfg=cfg, Bh=Bh, chunk_w=chunk_w, seg_off=seg_off)
    return plan, idxw, xpt, xot, own_nodes, deg


def build_program(plan, reps=1):
    cfg: Cfg = plan["cfg"]
    K, CS, T, KH, HK = cfg.K, cfg.CS, cfg.T, cfg.KH, cfg.HK
    Bh, chunk_w, seg_off = plan["Bh"], plan["chunk_w"], plan["seg_off"]
    NP = T * 128
    RANKS = CS // 128                                # 98
    W_max = int(max(chunk_w))
    OGRP = 4                                         # tiles per output DMA

    nc = bacc.Bacc(None, target_bir_lowering=False, num_swdge_queues=4)
    xpt = nc.declare_dram_parameter("xpt", [CH, K * CS], BF16, isOutput=False)
    xot_d = nc.declare_dram_parameter("xot", [CH, NP], BF16, isOutput=False)
    wth = nc.declare_dram_parameter("w_theta_t", [CH, CH], BF16, isOutput=False)
    wc_d = nc.declare_dram_parameter("w_c_t", [CH, CH], BF16, isOutput=False)
    wpn_d = nc.declare_dram_parameter("w_phi_tn", [CH, CH], BF16, isOutput=False)
    sentr = nc.declare_dram_parameter("sentr", [1, CH], BF16, isOutput=False)
    idxd = [nc.declare_dram_parameter(
        f"idx{k}", [128, max(int(chunk_w[k]) // 16, 16)], I16, isOutput=False)
        for k in range(K)]
    out = nc.declare_dram_parameter("out", [NP, CH], F32, isOutput=True)

    qrot = [0]

    with tile.TileContext(nc) as tc:
        with ExitStack() as ctx:
            consts = ctx.enter_context(tc.tile_pool(name="consts", bufs=1))
            dram = ctx.enter_context(tc.tile_pool(name="dram", bufs=1, space="DRAM"))
            xs = ctx.enter_context(tc.tile_pool(name="xs", bufs=2))
            ys = ctx.enter_context(tc.tile_pool(name="ys", bufs=1))
            idxp = ctx.enter_context(tc.tile_pool(name="idxp", bufs=1))
            ps_a = ctx.enter_context(tc.tile_pool(name="ps_a", bufs=2, space="PSUM"))
            ps_c = ctx.enter_context(tc.tile_pool(name="ps_c", bufs=2, space="PSUM"))
            gdst = ctx.enter_context(tc.tile_pool(name="gdst", bufs=4))
            accp = ctx.enter_context(tc.tile_pool(name="accp", bufs=1))
            finp = ctx.enter_context(tc.tile_pool(name="finp", bufs=2))

            y_hbm = dram.tile([K * CS, CH], BF16)

            wth_sb = consts.tile([CH, CH], BF16)
            nc.sync.dma_start(out=wth_sb[:], in_=wth[:])
            wc_sb = consts.tile([CH, CH], BF16)
            nc.sync.dma_start(out=wc_sb[:], in_=wc_d[:])
            wpn_sb = consts.tile([CH, CH], BF16)
            nc.sync.dma_start(out=wpn_sb[:], in_=wpn_d[:])
            xot = consts.tile([CH, NP], BF16)
            nc.sync.dma_start(out=xot[:], in_=xot_d[:])
            ident = consts.tile([128, 128], BF16)
            make_identity(nc, ident[:])

            for _rep in range(reps):
                acc = accp.tile([128, T * CH], BF16, tag="acc")
                for h in range(HK):
                    its = {}
                    for kk in range(KH):
                        k = h * KH + kk
                        # ---- phase A chunk k -> y_hbm rows (bf16)
                        HCS = (RANKS // 2) * 128     # half-chunk cols
                        ystage = ys.tile([128, CS], BF16, tag="ystage")
                        xh = [None, None]
                        for hf in range(2):
                            c0 = hf * HCS
                            cw = HCS if hf == 0 else CS - HCS
                            xh[hf] = xs.tile([128, (CS + 1) // 2], BF16,
                                             tag="xk", name=f"xh{hf}")
                            nc.sync.dma_start(
                                out=xh[hf][:, :cw],
                                in_=xpt[:, k * CS + c0: k * CS + c0 + cw])
                        for r0 in range(0, RANKS, 4):
                            rn = min(4, RANKS - r0)
                            pa = ps_a.tile([128, 512], F32, tag="pa")
                            for i in range(rn):
                                r = r0 + i
                                hf = 1 if r * 128 >= HCS else 0
                                cb = r * 128 - hf * HCS
                                nc.tensor.matmul(
                                    out=pa[:, i * 128: (i + 1) * 128],
                                    lhsT=xh[hf][:, cb: cb + 128],
                                    rhs=wth_sb[:], start=True, stop=True)
                            nc.scalar.copy(
                                out=ystage[:, r0 * 128: (r0 + rn) * 128],
                                in_=pa[:, : rn * 128])
                        # sentinel token CAPR = (partition 127, rank 97)
                        nc.sync.dma_start(
                            out=ystage[127:128, (RANKS - 1) * 128: RANKS * 128],
                            in_=sentr[:])
                        nc.sync.dma_start(
                            out=y_hbm[k * CS: (k + 1) * CS, :].rearrange(
                                "(p r) c -> p (r c)", p=128),
                            in_=ystage[:])
                        # idx for chunk k
                        wk = int(chunk_w[k])
                        it = idxp.tile([128, max(W_max // 16, 16)], I16,
                                       tag=f"it{kk}", name=f"it{k}")
                        if wk:
                            nc.sync.dma_start(out=it[:, : wk // 16],
                                              in_=idxd[k][:, : wk // 16])
                        its[kk] = it

                    # ---- gathers + in-place fold + merge per tile
                    for t in range(T):
                        bh = int(Bh[t, h])
                        if bh == 0:
                            continue
                        nb = KH * bh
                        dst = gdst.tile([128, KH * int(Bh[:, h].max()) * CH],
                                        BF16, tag="dst")
                        dst3 = dst[:].rearrange("p (b c) -> p b c", c=CH)
                        for kk in range(KH):
                            k = h * KH + kk
                            so = int(seg_off[t, k])
                            it = its[kk]
                            for b0 in range(0, bh, 8):
                                bn = min(8, bh - b0)
                                ni = 128 * bn
                                o0 = so + b0 * 128
                                nc.gpsimd.dma_gather(
                                    out_ap=dst3[:, kk * bh + b0:
                                                kk * bh + b0 + bn, :],
                                    in_ap=y_hbm[k * CS: (k + 1) * CS, :],
                                    idxs_ap=it[:, o0 // 16: (o0 + ni) // 16],
                                    num_idxs=ni, num_idxs_reg=ni,
                                    elem_size=CH,
                                    queue_num=qrot[0] % 4)
                                qrot[0] += 1
                        # in-place fold tree on dst
                        nbl = nb
                        while nbl > 1:
                            half = (nbl + 1) // 2
                            nc.vector.tensor_tensor(
                                out=dst[:, : half * CH],
                                in0=dst[:, : half * CH],
                                in1=dst[:, (nbl - half) * CH: nbl * CH],
                                op=mybir.AluOpType.min)
                            nbl = half
                        if h == 0:
                            nc.vector.tensor_copy(
                                out=acc[:, t * CH: (t + 1) * CH],
                                in_=dst[:, :CH])
                        else:
                            nc.vector.tensor_tensor(
                                out=acc[:, t * CH: (t + 1) * CH],
                                in0=acc[:, t * CH: (t + 1) * CH],
                                in1=dst[:, :CH], op=mybir.AluOpType.min)

                # ---- phase C: out = x_own @ Wc.T - m @ Wphi.T
                for t0 in range(0, T, OGRP):
                    tn = min(OGRP, T - t0)
                    ost = finp.tile([128, OGRP * CH], F32, tag="ost")
                    pt = ps_c.tile([128, OGRP * CH], BF16, tag="pt")
                    for i in range(tn):
                        t = t0 + i
                        nc.tensor.transpose(
                            out=pt[:, i * CH: (i + 1) * CH],
                            in_=acc[:, t * CH: (t + 1) * CH],
                            identity=ident[:])
                    mt = finp.tile([128, OGRP * CH], BF16, tag="mt")
                    nc.scalar.copy(out=mt[:, : tn * CH], in_=pt[:, : tn * CH])
                    po = ps_c.tile([128, OGRP * CH], F32, tag="po")
                    for i in range(tn):
                        t = t0 + i
                        nc.tensor.matmul(
                            out=po[:, i * CH: (i + 1) * CH],
                            lhsT=xot[:, t * 128: (t + 1) * 128],
                            rhs=wc_sb[:], start=True, stop=False)
                        nc.tensor.matmul(
                            out=po[:, i * CH: (i + 1) * CH],
                            lhsT=mt[:, i * CH: (i + 1) * CH],
                            rhs=wpn_sb[:], start=False, stop=True)
                    nc.scalar.copy(out=ost[:, : tn * CH], in_=po[:, : tn * CH])
                    # out rows partition-major: row p*T + t
                    nc.sync.dma_start(
                        out=out[:].rearrange("(p t) c -> p t c", p=128)[
                            :, t0: t0 + tn, :],
                        in_=ost[:, : tn * CH].rearrange(
                            "p (i c) -> p i c", c=CH))
    nc.compile()
    return nc


_CACHE = {}


def run_cfg(x, edge_index, W_theta, W_phi, cfg: Cfg, time_iters=0, reps=1):
    ck = (np.asarray(edge_index)[0, :64].tobytes(),
          np.asarray(edge_index)[1, :64].tobytes(), cfg)
    hit = _CACHE.get("prep")
    if hit is not None and hit[0] == ck:
        plan, idxw, xpt, xot, own_nodes, deg = hit[1]
    else:
        plan, idxw, xpt, xot, own_nodes, deg = prep(x, edge_index, cfg)
        _CACHE["prep"] = (ck, (plan, idxw, xpt, xot, own_nodes, deg))

    skey = (cfg, reps, tuple(plan["Bh"].reshape(-1).tolist()))
    if skey not in _CACHE:
        _CACHE[skey] = BassRunner(build_program(plan, reps=reps), cfg.n_cores)
    runner = _CACHE[skey]

    wt = np.asarray(W_theta, np.float32)
    wp = np.asarray(W_phi, np.float32)
    wtt = np.ascontiguousarray(wt.T).astype(ml_dtypes.bfloat16)
    wct = np.ascontiguousarray((wp @ wt).T).astype(ml_dtypes.bfloat16)
    wpn = np.ascontiguousarray(-wp.T).astype(ml_dtypes.bfloat16)
    sentr = np.full((1, CH), SENT, np.float32).astype(ml_dtypes.bfloat16)
    in_maps = []
    for c in range(cfg.n_cores):
        m = dict(xpt=xpt, xot=np.ascontiguousarray(xot[c]),
                 w_theta_t=wtt, w_c_t=wct, w_phi_tn=wpn, sentr=sentr)
        for k in range(cfg.K):
            w = idxw[k][c]
            if w.shape[1] == 0:
                w = np.zeros((128, 16), np.int16)
            m[f"idx{k}"] = np.ascontiguousarray(w)
        in_maps.append(m)
    runner.prepare(in_maps)
    outs = runner.run()
    t_ns = runner.time_ns(iters=time_iters) if time_iters else None
    res = runner.results(outs)
    out_full = np.empty((cfg.N, CH), np.float32)
    T = cfg.T
    for c in range(cfg.n_cores):
        # device out rows are partition-major: row p*T + t -> node pos t*128+p
        o = res[c]["out"].reshape(128, T, CH).transpose(1, 0, 2).reshape(-1, CH)
        out_full[own_nodes[c]] = o[: cfg.npc]
    out_full[deg == 0] = 0.0
    return out_full, t_ns


def kernel(x, edge_index, W_theta, W_phi):
    out, _ = run_cfg(x, edge_index, W_theta, W_phi, Cfg())
    return out
